# revision 82
# baseline (speedup 1.0000x reference)
"""Trainium2 Bass kernel for nn_MultiHeadAttention (B=4, T=1024, D=1024, H=16, dk=64).

Sharding: 8 cores = 4 batches x 2 head-groups (8 heads / 512 features each).
Each core computes a partial output (its head-group's contribution through Wo);
host sums the two partials per batch and adds bo.

Per-core dataflow (one NeuronCore, Tile-scheduled), cost-model-guided; all
matmuls bf16 (fp8 DoubleRow was tried and rejected: e4m3's 2.65% RMS lands
~6% on the output, over the 2e-2 gate):
  A) q/k projections (bf16, 8 d-chunk accumulation) -> LN per head
     (bn_stats + bn_aggr, Rsqrt(var+eps) on ACT) -> normalize (Pool) ->
     qh bf16 -> 4 PE transposes per tile into a [P,512] PSUM quad ->
     one ACT scale-drain (gamma, and 1/sqrt(dk) on q) into qlnT/klnT
     [pair-features x 4 pairs x T].
  B) head-major attention: scoresT per (h, tk-block) = klnT-slice.T @ qlnT
     (2x 512-chunk matmuls, K=64) -> PSUM [128,1024]; exp on ACT (the
     bottleneck engine of this phase - nothing else is scheduled on ACT
     here); mask multiply on DVE; attnV transposed: out partitions = query
     positions, rhs = v_sb[tk,h,:] = [v|1] (65 cols, ones column yields
     softmax denominators in col 64) accumulated over tk chunks into
     bank-sized PSUM [128,4,128] tiles; DVE reciprocal + per-qb
     tensor_scalar -> xT bf16; after each odd head 8 PE transposes ->
     x_all [f, T]. The v-projection streams as 4-matmul chunks through the
     first 16 units (head 0's attnV lags 9 units so v(tk) always lands in
     time), filling the ACT-bound window with PE work.
  C) out = x_all-slices.T @ Wo (bf16) -> (T,1024) bf16 partial, DMA out
     (alternating sync/scalar HWDGE queues; host sums partials in f32).

PSUM (8 banks): pp 2 (proj/v/out + B-phase x-transpose quads), sc 2x2
(scores [P,1024]; also hosts phase-A transpose quads - disjoint phases),
xps 2 (attnV accumulators). start=True zeroes a whole 2KB bank, so
multi-region accumulation uses start only on the globally-first matmul and
stop on the last; skip_group_check silences the checker for interior ones.

walrus allows only ONE sync-wait per instruction (_split_excess_waits
patches the BIR).
"""

import numpy as np
import ml_dtypes

T = 1024
D = 1024
F = 512      # features per core (8 heads x 64)
NH = 8       # heads per core
DK = 64
P = 128
EPS = 1e-5
BF16 = ml_dtypes.bfloat16

_CACHE = {}

# tuning knobs
PP_BUFS = 2
SC_BUFS = 2
XPS_BUFS = 2
AT_BUFS = 13
XT_BUFS = 3
DRAIN_BUFS = 8
QH_BUFS = 8
STAT_BUFS = 13
OUT_BUFS = 6
DEFER_PAIR_TRANSPOSE = True  # emit pair transposes after next head's first scores


def _split_excess_waits(bj):
    """Walrus allows at most 1 sync-wait per instruction (2 for
    EventSemaphore). Tile's sem assigner can emit more; spill the excess
    onto NoOp carriers inserted just before, on the same engine."""
    import json
    d = json.loads(bj)
    ctr = 0
    for fn in d["functions"]:
        for bb in fn["blocks"]:
            new = []
            for inst in bb["instructions"]:
                si = inst.get("sync_info") or {}
                ow = si.get("on_wait") or []
                op = inst.get("opcode", "")
                cap = 2 if op == "EventSemaphore" else 1
                if len(ow) > cap:
                    for w in ow[:-cap]:
                        ctr += 1
                        new.append({
                            "debug": inst.get("debug", 0),
                            "engine": inst["engine"],
                            "ins": [], "outs": [],
                            "name": f"W-{ctr}",
                            "opcode": "NoOp",
                            "sync_info": {"on_update": [], "on_wait": [w]},
                            "text_hint": "waitsplit",
                        })
                    si["on_wait"] = ow[-cap:]
                new.append(inst)
            bb["instructions"] = new
    return json.dumps(d).encode(), ctr


def _build(use_bq, use_bk, use_bv, ln_beta_zero=True):
    import concourse.bass as bass
    import concourse.tile as tile
    from concourse import mybir
    from concourse.masks import make_identity

    f32 = mybir.dt.float32
    bf16 = mybir.dt.bfloat16
    ALU = mybir.AluOpType
    ACTF = mybir.ActivationFunctionType

    nc = bass.Bass()

    # ---- DRAM I/O ----
    xq_d = nc.dram_tensor("xq16", (D, T), bf16, kind="ExternalInput").ap()
    xk_d = nc.dram_tensor("xk16", (D, T), bf16, kind="ExternalInput").ap()
    xv_d = nc.dram_tensor("xv16", (D, T), bf16, kind="ExternalInput").ap()
    wq_d = nc.dram_tensor("wq16", (D, F), bf16, kind="ExternalInput").ap()
    wk_d = nc.dram_tensor("wk16", (D, F), bf16, kind="ExternalInput").ap()
    wv_d = nc.dram_tensor("wv16", (D, F), bf16, kind="ExternalInput").ap()
    wo_d = nc.dram_tensor("wo16", (F, D), bf16, kind="ExternalInput").ap()
    mask_d = nc.dram_tensor("mask16", (T, T), bf16, kind="ExternalInput").ap()
    # per-partition LN constants (128,1) = per pair-local feature
    gl_d = {}
    for nm in ("gq", "gk"):
        gl_d[nm] = nc.dram_tensor(nm, (P, 1), f32, kind="ExternalInput").ap()
    if not ln_beta_zero:
        for nm in ("bq_ln", "bk_ln"):
            gl_d[nm] = nc.dram_tensor(nm, (P, 1), f32, kind="ExternalInput").ap()
    biases = {}
    for name, used in (("bq", use_bq), ("bk", use_bk), ("bv", use_bv)):
        if used:
            biases[name] = nc.dram_tensor(name, (F,), f32, kind="ExternalInput").ap()
    out_p = nc.dram_tensor("out_p", (T, D), bf16, kind="ExternalOutput").ap()

    # DRAM views
    xviews = {
        "q": xq_d.rearrange("(dc p) t -> p dc t", p=P),
        "k": xk_d.rearrange("(dc p) t -> p dc t", p=P),
        "v": xv_d.rearrange("(dc p) t -> p dc t", p=P),
    }
    wviews = {
        "q": wq_d.rearrange("(dc p) f -> p dc f", p=P),
        "k": wk_d.rearrange("(dc p) f -> p dc f", p=P),
        "v": wv_d.rearrange("(dc p) f -> p dc f", p=P),
    }
    wo_view = wo_d.rearrange("(fc p) d -> p fc d", p=P)
    mask_view = mask_d.rearrange("(kc p) t -> p kc t", p=P)
    out_view = out_p.rearrange("(tc p) d -> p tc d", p=P)

    with tile.TileContext(nc) as tc:
        with (
            tc.tile_pool(name="const", bufs=1) as const,
            tc.tile_pool(name="drain", bufs=DRAIN_BUFS) as drain,
            tc.tile_pool(name="stat", bufs=STAT_BUFS) as stat,
            tc.tile_pool(name="qhatp", bufs=QH_BUFS) as qhatp,
            tc.tile_pool(name="attnp", bufs=AT_BUFS) as attnp,
            tc.tile_pool(name="xtp", bufs=XT_BUFS) as xtp,
            tc.tile_pool(name="recipp", bufs=4) as recipp,
            tc.tile_pool(name="outp", bufs=OUT_BUFS) as outp,
            tc.tile_pool(name="psum", bufs=1, space="PSUM") as psum,
        ):
            def pp_tile(shape=(P, F), dtype=f32, name="pp"):
                return psum.tile(list(shape), dtype, name=name, tag="pp",
                                 bufs=PP_BUFS)

            def sc_tile(shape=(P, T), dtype=f32, name="sc"):
                return psum.tile(list(shape), dtype, name=name, tag="sc",
                                 bufs=SC_BUFS)

            def xps_tile(g):
                return psum.tile([P, 4, P], f32, name=f"xps{g}", tag="xps",
                                 bufs=XPS_BUFS)

            # ---- PE p-state warm-up ----
            # The PE needs early, continuous work to ramp to 2.4GHz; it
            # otherwise idles on input DMAs and crawls through the first
            # tiles at 1.2GHz (and a post-idle LOW-state quirk). Identity
            # first (before the SWDGE descriptor-gens hog the Pool engine),
            # then bridge the DMA wait with throwaway matmuls.
            warm = const.tile([P, F], bf16, name="warm", tag="warm")
            nc.vector.memset(warm, 0.5)
            for _ in range(14):
                wps = sc_tile((P, F), f32, name="wps")
                nc.tensor.matmul(wps, lhsT=warm[:, 0:P], rhs=warm,
                                 start=True, stop=True)
            ident16 = const.tile([P, P], bf16, name="ident16", tag="ident16")
            make_identity(nc, ident16)

            # ---- resident tiles ----
            x_sb = {
                pn: const.tile([P, 8, T], bf16, name=f"x{pn}_sb", tag=f"x{pn}_sb")
                for pn in ("q", "k", "v")
            }
            w_sb = {
                pn: const.tile([P, 8, F], bf16, name=f"w{pn}_sb", tag=f"w{pn}_sb")
                for pn in ("q", "k", "v")
            }
            wo_sb = const.tile([P, 4, D], bf16, name="wo", tag="wo")
            mask_sb = const.tile([P, 8, T], bf16, name="mask", tag="mask")
            qlnT = const.tile([P, 4, T], bf16, name="qlnT", tag="qlnT")
            klnT = const.tile([P, 4, T], bf16, name="klnT", tag="klnT")
            v_sb = const.tile([P, 8, NH, 65], bf16, name="v_sb", tag="v_sb")
            x_all = const.tile([P, 4, T], bf16, name="x_all", tag="x_all")
            eps_t = const.tile([P, 1], f32, name="eps", tag="eps")
            gb_t = {}
            for nm, dr_ in gl_d.items():
                gb_t[nm] = const.tile([P, 1], f32, name=f"ln_{nm}", tag=f"ln_{nm}")
                nc.gpsimd.dma_start(gb_t[nm], dr_)
            nc.vector.memset(eps_t, EPS)
            # ones column for softmax denominators
            nc.vector.memset(v_sb[:, :, :, 64:65], 1.0)


            bias_bc = {}
            for name in biases:
                bias_bc[name] = const.tile([P, F], f32, name=f"bc_{name}", tag=f"bc_{name}")
                src = bass.AP(
                    tensor=biases[name].tensor,
                    offset=biases[name].offset,
                    ap=[[0, P], [1, F]],
                )
                nc.gpsimd.dma_start(out=bias_bc[name], in_=src)

            # ---- input DMAs ----
            # sync queue, ordered so the k-projection can start ASAP
            def load_x(pn, quarters, eng=None):
                for qtr in quarters:
                    sl = slice(qtr * 256, (qtr + 1) * 256)
                    (eng or nc.sync).dma_start(
                        x_sb[pn][:, :, sl], xviews[pn][:, :, sl])

            nc.sync.dma_start(w_sb["k"][:, 0:4, :], wviews["k"][:, 0:4, :])
            load_x("k", range(1))
            nc.sync.dma_start(w_sb["k"][:, 4:8, :], wviews["k"][:, 4:8, :])
            load_x("k", range(1, 4))
            nc.sync.dma_start(w_sb["q"], wviews["q"])
            load_x("q", range(4))
            nc.sync.dma_start(w_sb["v"], wviews["v"])
            load_x("v", range(4))
            nc.sync.dma_start(wo_sb, wo_view)
            # mask via the SWDGE queue, concurrent with the sync queue
            for half in range(2):
                nc.gpsimd.dma_start(mask_sb[:, 4 * half:4 * half + 4, :],
                                    mask_view[:, 4 * half:4 * half + 4, :])

            # ---- Phase A: q/k projections + LN + transpose ----
            a_pending = []

            def flush_a():
                while a_pending:
                    a_pending.pop(0)()

            def proj_ln(pn, dstT, t):
                bias_name = "b" + pn
                ps = pp_tile()
                for d in range(8):
                    nc.tensor.matmul(
                        ps, lhsT=x_sb[pn][:, d, t * P:(t + 1) * P],
                        rhs=w_sb[pn][:, d, :],
                        start=(d == 0), stop=(d == 7),
                    )
                # deferred transposes/gamma-drains of older tiles go here:
                # after this tile's matmuls (PE) and before its drain (ACT),
                # so neither engine's in-order queue blocks them.
                while len(a_pending) > 6:
                    a_pending.pop(0)()
                sb = drain.tile([P, NH, DK], f32, name="qsb", tag="qsb")
                if bias_name in bias_bc:
                    nc.vector.tensor_add(
                        sb.rearrange("p h d -> p (h d)"), ps, bias_bc[bias_name])
                else:
                    nc.scalar.activation(
                        out=sb.rearrange("p h d -> p (h d)"), in_=ps,
                        func=ACTF.Copy)
                st = stat.tile([P, NH, 6], f32, name="st", tag="st")
                for h in range(NH):
                    nc.vector.bn_stats(out=st[:, h, :], in_=sb[:, h, :])
                ag = stat.tile([P, NH, 2], f32, name="ag", tag="ag")
                for h in range(NH):
                    nc.vector.bn_aggr(out=ag[:, h, :], in_=st[:, h, :])
                sd = stat.tile([P, NH], f32, name="sd", tag="sd")
                nc.scalar.activation(
                    out=sd, in_=ag[:, :, 1], func=ACTF.Sqrt, bias=eps_t)
                rs = stat.tile([P, NH], f32, name="rs", tag="rs")
                nc.vector.reciprocal(out=rs, in_=sd)
                qh = qhatp.tile([P, F], bf16, name="qh", tag="qh")
                for h in range(NH):
                    nc.gpsimd.tensor_scalar(
                        out=qh[:, h * DK:(h + 1) * DK],
                        in0=sb[:, h, :],
                        scalar1=ag[:, h, 0:1],
                        scalar2=rs[:, h:h + 1],
                        op0=ALU.subtract,
                        op1=ALU.mult,
                    )
                # 4 pair-transposes into one [P,512] PSUM quad (on the sc
                # tag: scores don't run during phase A), then ONE gamma
                # scale-drain. Deferred one tile so the PE doesn't wait on
                # the LN chain.
                def emit(qh=qh, pn=pn, dstT=dstT, t=t):
                    pst4 = sc_tile((P, 4, P), bf16, name="pst4")
                    for j in range(4):
                        nc.tensor.transpose(
                            pst4[:, j, :], qh[:, j * P:(j + 1) * P], ident16)
                    g_nm, b_nm = ("gq", "bq_ln") if pn == "q" else ("gk", "bk_ln")
                    dst = dstT[:, :, t * P:(t + 1) * P]
                    if ln_beta_zero:
                        nc.scalar.activation(
                            out=dst, in_=pst4, func=ACTF.Copy, scale=gb_t[g_nm])
                    else:
                        nc.scalar.tensor_scalar(
                            out=dst, in0=pst4,
                            scalar1=gb_t[g_nm], scalar2=gb_t[b_nm],
                            op0=ALU.mult, op1=ALU.add)
                a_pending.append(emit)

            order = [("k", klnT, t) for t in range(8)] + \
                [("q", qlnT, t) for t in range(8)]
            for pn, dstT, t in order:
                proj_ln(pn, dstT, t)
            while a_pending:
                a_pending.pop(0)()

            # ---- Phase B: attention, one flat software pipeline ----
            # Per unit (h, tk): emit scores/exp/mask, then the PREVIOUS
            # unit's attnV matmuls, so the PE never waits on exp+mask.
            # Head drains (reciprocal + scale) and pair transposes are
            # emitted when that head's last attnV retires.
            pending = []  # deferred pair-transpose emitters

            def flush_pending():
                while pending:
                    pending.pop(0)()

            xps_h = {}
            xTb_h = {}

            def attn_v(h, tk, at):
                xps = xps_h[h]
                for qg in range(2):
                    for qb in range(4):
                        j = qg * 4 + qb
                        first = (tk == 0 and qb == 0)
                        last = (tk == 7 and qb == 3)
                        nc.tensor.matmul(
                            xps[qg][:, qb, 0:65],
                            lhsT=at[:, j * P:(j + 1) * P],
                            rhs=v_sb[:, tk, h, :],
                            start=first, stop=last,
                            skip_group_check=not (first or last),
                        )

            def head_drain(h):
                xps = xps_h.pop(h)
                if h % 2 == 0:
                    xTb_h[h // 2] = xtp.tile([P, 8, P], bf16, name="xTb", tag="xTb")
                xTb = xTb_h[h // 2]
                csl = slice(0, DK) if h % 2 == 0 else slice(DK, P)
                for qg in range(2):
                    rc = recipp.tile([P, 4], f32, name="rc", tag="rc")
                    nc.vector.reciprocal(out=rc, in_=xps[qg][:, :, 64:65])
                    for qb in range(4):
                        nc.vector.tensor_scalar(
                            out=xTb[:, qg * 4 + qb, csl],
                            in0=xps[qg][:, qb, 0:64],
                            scalar1=rc[:, qb:qb + 1], scalar2=None,
                            op0=ALU.mult)
                if h % 2 == 1:
                    jj = h // 2

                    def emit_transposes(xTb=xTb, jj=jj):
                        for qg in range(2):
                            pst4 = pp_tile((P, 4, P), bf16, name="pstx")
                            for qb in range(4):
                                nc.tensor.transpose(
                                    pst4[:, qb, :], xTb[:, qg * 4 + qb, :], ident16)
                            nc.vector.tensor_copy(
                                out=x_all[:, jj, qg * F:(qg + 1) * F],
                                in_=pst4.rearrange("p a b -> p (a b)"))
                    if DEFER_PAIR_TRANSPOSE:
                        pending.append(emit_transposes)
                    else:
                        emit_transposes()

            at_q = []
            v_tasks = []  # (tk, d_lo, d_hi, drain?) chunks, 4 matmuls each
            for tk in range(8):
                v_tasks.append((tk, 0, 4, False))
                v_tasks.append((tk, 4, 8, True))
            v_ps = {}

            def v_chunk():
                tk, dlo, dhi, do_drain = v_tasks.pop(0)
                if dlo == 0:
                    v_ps[tk] = pp_tile()
                ps = v_ps[tk]
                for d in range(dlo, dhi):
                    nc.tensor.matmul(
                        ps, lhsT=x_sb["v"][:, d, tk * P:(tk + 1) * P],
                        rhs=w_sb["v"][:, d, :],
                        start=(d == 0), stop=(d == 7),
                    )
                if do_drain:
                    ps = v_ps.pop(tk)
                    if "bv" in bias_bc:
                        vb = drain.tile([P, NH, DK], f32, name="vsb", tag="qsb")
                        nc.vector.tensor_add(
                            vb.rearrange("p h d -> p (h d)"), ps, bias_bc["bv"])
                        nc.gpsimd.tensor_copy(out=v_sb[:, tk, :, 0:64], in_=vb)
                    else:
                        nc.vector.tensor_copy(
                            out=v_sb[:, tk, :, 0:64],
                            in_=ps.rearrange("p (h c) -> p h c", c=DK))

            def pop_attnv():
                hp, tkp, atp_ = at_q.pop(0)
                attn_v(hp, tkp, atp_)
                if tkp == 7:
                    head_drain(hp)
                if hp % 2 == 1 and tkp == 1:
                    flush_pending()

            u = 0
            for h in range(NH):
                rows = slice((h % 2) * DK, (h % 2) * DK + DK)
                pair = h // 2
                xps_h[h] = [xps_tile(g) for g in range(2)]
                for tk in range(8):
                    # v-projection: one 4-matmul chunk per unit over the
                    # first 16 units; h=0's attnV lags 9 units so v(tk) is
                    # always emitted before its consumer is popped
                    if v_tasks:
                        v_chunk()
                    sp = sc_tile()
                    for n in range(2):
                        nc.tensor.matmul(
                            sp[:, n * F:(n + 1) * F],
                            lhsT=klnT[rows, pair, tk * P:(tk + 1) * P],
                            rhs=qlnT[rows, pair, n * F:(n + 1) * F],
                            start=True, stop=True,
                        )
                    at = attnp.tile([P, T], bf16, name="at", tag="at")
                    nc.scalar.activation(out=at, in_=sp, func=ACTF.Exp)
                    nc.vector.tensor_mul(at, at, mask_sb[:, tk, :])
                    at_q.append((h, tk, at))
                    target = 9 if u < 20 else max(1, 9 - (u - 20) // 3)
                    while len(at_q) > target:
                        pop_attnv()
                    u += 1
            while at_q:
                pop_attnv()
            # the last pair's transposes stay pending; phase C's first unit
            # flushes them between its jj=0-2 and jj=3 accumulation

            # ---- Phase C: output projection ----
            for t in range(8):
                for n in range(2):
                    r = (2 * t + n) % 3
                    if r == 0:
                        ps = pp_tile()
                    elif r == 1:
                        ps = sc_tile((P, F), f32, name="scc")
                    else:
                        ps = psum.tile([P, F], f32, name="xpc", tag="xps",
                                       bufs=XPS_BUFS)
                    for jj in range(4):
                        if t == 0 and n == 0 and jj == 3:
                            # first unit: pairs 0-2 accumulate while the last
                            # pair's x-transposes (deferred above) drain; its
                            # jj=3 term lands right after their flush
                            flush_pending()
                        nc.tensor.matmul(
                            ps, lhsT=x_all[:, jj, t * P:(t + 1) * P],
                            rhs=wo_sb[:, jj, n * F:(n + 1) * F],
                            start=(jj == 0), stop=(jj == 3),
                        )
                    ob = outp.tile([P, F], bf16, name="ob", tag="ob")
                    nc.scalar.activation(out=ob, in_=ps, func=ACTF.Copy)
                    nc.sync.dma_start(out=out_view[:, t, n * F:(n + 1) * F], in_=ob)

    return nc


def _get_nc(flags):
    if len(flags) == 3:
        flags = (*flags, True)
    key = flags
    if key not in _CACHE:
        nc = _build(*flags)
        patched, _n = _split_excess_waits(nc.to_json_bytes())
        nc.to_json_bytes = lambda: patched
        _CACHE[key] = nc
    return _CACHE[key]


def _bf(a):
    return np.ascontiguousarray(np.asarray(a).astype(BF16))


def kernel(query, key, value, mask, Wq, bq, Wk, bk, Wv, bv, Wo, bo,
           q_gamma, q_beta, k_gamma, k_beta, _trace=False):
    from concourse.bass_utils import run_bass_kernel_spmd

    query = np.asarray(query, np.float32)
    key = np.asarray(key, np.float32)
    value = np.asarray(value, np.float32)
    mask = np.asarray(mask)
    Wq, Wk, Wv, Wo = (np.asarray(w, np.float32) for w in (Wq, Wk, Wv, Wo))
    bq, bk, bv, bo = (np.asarray(b, np.float32) for b in (bq, bk, bv, bo))
    q_gamma, q_beta, k_gamma, k_beta = (
        np.asarray(g, np.float32) for g in (q_gamma, q_beta, k_gamma, k_beta))

    B = query.shape[0]
    use_bq, use_bk, use_bv = (bool(np.any(b)) for b in (bq, bk, bv))
    ln_beta_zero = not (np.any(q_beta) or np.any(k_beta))
    nc = _get_nc((use_bq, use_bk, use_bv, ln_beta_zero))

    # host-side shard prep
    xq16 = [_bf(query[b].T) for b in range(B)]
    xk16 = [_bf(key[b].T) for b in range(B)]
    xv16 = [_bf(value[b].T) for b in range(B)]
    mask16 = [np.ascontiguousarray((~mask[b]).T.astype(BF16)) for b in range(B)]
    # per-partition LN consts (pair-local feature); q folds 1/sqrt(dk)=1/8
    def tile2(v):
        return np.ascontiguousarray(np.tile(v, 2).reshape(P, 1).astype(np.float32))
    consts = {
        "gq": tile2(q_gamma / 8.0),
        "gk": tile2(k_gamma),
    }
    if not ln_beta_zero:
        consts.update({
            "bq_ln": tile2(q_beta / 8.0),
            "bk_ln": tile2(k_beta),
        })

    in_maps = []
    for core in range(8):
        b, g = core // 2, core % 2
        sl = slice(g * F, (g + 1) * F)
        im = {
            "xq16": xq16[b], "xk16": xk16[b], "xv16": xv16[b],
            "wq16": _bf(Wq[sl].T),
            "wk16": _bf(Wk[sl].T),
            "wv16": _bf(Wv[sl].T),
            "wo16": _bf(Wo[:, sl].T),
            "mask16": mask16[b],
            **consts,
        }
        if use_bq:
            im["bq"] = np.ascontiguousarray(bq[sl])
        if use_bk:
            im["bk"] = np.ascontiguousarray(bk[sl])
        if use_bv:
            im["bv"] = np.ascontiguousarray(bv[sl])
        in_maps.append(im)

    res = run_bass_kernel_spmd(nc, in_maps, core_ids=list(range(8)), trace=_trace)
    out = np.zeros((B, T, D), np.float32)
    for b in range(B):
        out[b] = (res.results[2 * b]["out_p"].astype(np.float32)
                  + res.results[2 * b + 1]["out_p"].astype(np.float32) + bo)
    if _trace:
        kernel._last_results = res
    return out
